# revision 1
# baseline (speedup 1.0000x reference)
# Multi-head attention kernel for Trainium2, sharded over 8 NeuronCores.
#
# Sharding: core = (batch b, query-chunk qc). Each core handles QB=512 queries
# of one batch, all 12 heads, recomputing the K/V projections for its batch
# (cheaper than cross-core collectives on this chip).
#
# Layout strategy (bf16 matmul operands, fp32 PSUM accumulation/epilogues):
#   - Host pre-transposes activations to [E, S] so the contraction dim (E)
#     lands on SBUF partitions; fp32 matmul is avoided on-device (it lowers to
#     two PE passes), so all matmul operands are bf16.
#   - q^T, k^T computed as [768, S] via lhsT=W chunks; per-partition bias
#     added during the PSUM->SBUF copy (DVE tensor_scalar, bf16 output).
#   - v computed directly as [keys, 768] using x_v^T chunks as the stationary
#     operand; stored with a ones-column per head ([128,16,12,65]) so the PV
#     matmul (M=65) also produces the softmax denominator row for free.
#   - scores^T = [keys, queries] per head: K=64 matmuls; even/odd heads sit in
#     partition halves 0-63/64-127, emitted adjacently so they land in
#     disjoint PE row groups and run concurrently (row packing).
#   - exp on ScalarE in [128, 2x512] groups PSUM->SBUF (bf16), streamed
#     straight into the accumulating PV matmul (no full score matrix in SBUF).
#   - softmax normalize: per-pair staging copies + SBUF->SBUF gather of the 12
#     denominator rows, one batched DVE reciprocal, per-head PE broadcast
#     (selector matmul) and DVE multiply. Note: accumulating matmul groups
#     must not mix tile positions (HW), hence K=128 head-pair contractions in
#     the output projection; DVE ops need 32-aligned base partitions.
#   - output projection contracts head pairs as K=128 matmuls; epilogue adds
#     host-precomputed bias (bv folded through Wo + bo).

import numpy as np
from contextlib import ExitStack

import concourse.bass as bass
import concourse.mybir as mybir
import concourse.tile as tile
from concourse import bacc
from concourse.bass_utils import run_bass_kernel_spmd

F32 = mybir.dt.float32
BF16 = mybir.dt.bfloat16
P = 128
E = 768
S = 2048
B = 2
H = 12
D = 64
QB = 512          # queries per core
NCORES = 8
EC = E // P       # 6 e-chunks
KT = S // P       # 16 key tiles
MT_Q = E // P     # 6 M-tiles for q^T/k^T (768 rows)
NC4 = S // 512    # 4 n-slices of k^T


def build_nc():
    nc = bacc.Bacc("TRN2", debug=False)

    # DRAM I/O (per-core shapes; same NEFF on all 8 cores)
    xq = nc.dram_tensor("xq", (E, QB), BF16, kind="ExternalInput")     # query[b,chunk].T
    xk = nc.dram_tensor("xk", (E, S), BF16, kind="ExternalInput")      # key[b].T
    xv = nc.dram_tensor("xv", (E, S), BF16, kind="ExternalInput")      # value[b].T
    wq = nc.dram_tensor("wq", (E, E), BF16, kind="ExternalInput")      # [E, H*D], pre-scaled 1/sqrt(D)
    wk = nc.dram_tensor("wk", (E, E), BF16, kind="ExternalInput")
    wv = nc.dram_tensor("wv", (E, E), BF16, kind="ExternalInput")
    wo = nc.dram_tensor("wo", (E, E), BF16, kind="ExternalInput")
    bq = nc.dram_tensor("bq", (P, MT_Q), F32, kind="ExternalInput")   # per-partition bias per M-tile
    bk = nc.dram_tensor("bk", (P, MT_Q), F32, kind="ExternalInput")
    bo = nc.dram_tensor("bo", (P, E), F32, kind="ExternalInput")      # bv@Wo + bo, broadcast
    seld = nc.dram_tensor("seld", (H, H * D), F32, kind="ExternalInput")  # head-broadcast selector
    out = nc.dram_tensor("out", (QB, E), F32, kind="ExternalOutput")

    with tile.TileContext(nc) as tc:
        with ExitStack() as ctx:
            _emit(ctx, tc, nc, xq, xk, xv, wq, wk, wv, wo, bq, bk, bo, seld, out)
    nc.compile()
    return nc


def _emit(ctx, tc, nc, xq, xk, xv, wq, wk, wv, wo, bq, bk, bo, seld, out):
    # ---- pools ----
    # SBUF persistent
    persist = ctx.enter_context(tc.tile_pool(name="persist", bufs=1))
    # big weight slots [128, 6, 768] reused wq -> wk -> wv -> wo
    wpool = ctx.enter_context(tc.tile_pool(name="wpool", bufs=2))
    # x input slices
    xpool = ctx.enter_context(tc.tile_pool(name="xpool", bufs=2))
    xvpool = ctx.enter_context(tc.tile_pool(name="xvpool", bufs=3))
    # exp output stream
    epool = ctx.enter_context(tc.tile_pool(name="epool", bufs=4))
    # small temps
    spool = ctx.enter_context(tc.tile_pool(name="spool", bufs=2))
    outpool = ctx.enter_context(tc.tile_pool(name="outpool", bufs=2))
    # PSUM pools
    psA = ctx.enter_context(tc.tile_pool(name="psA", bufs=2, space="PSUM"))   # [128,512] proj qk + PV out
    psB = ctx.enter_context(tc.tile_pool(name="psB", bufs=1, space="PSUM"))   # [128,768] v proj + out proj
    psC = ctx.enter_context(tc.tile_pool(name="psC", bufs=2, space="PSUM"))   # [128,2,512] scores

    # ---- persistent SBUF tensors ----
    qT = persist.tile([P, MT_Q, QB], BF16)       # q^T [768, QB]
    kT = persist.tile([P, MT_Q, S], BF16)        # k^T [768, S]
    v_sb = persist.tile([P, KT, H, D + 1], BF16)  # v + ones column per head
    o_all = persist.tile([P, H // 2, QB], BF16)   # normalized o^T, head pairs in partition halves
    bq_sb = persist.tile([P, MT_Q], F32)
    bk_sb = persist.tile([P, MT_Q], F32)
    bo_sb = persist.tile([P, E], F32)
    o_raw = persist.tile([D + 1, H, 512], F32)   # unnormalized o^T + denom row per head
    dens = persist.tile([H, 512], F32)           # gathered denominators
    drec = persist.tile([H, 512], F32)           # their reciprocals
    sel = persist.tile([H, H * D], F32)          # selector: sel[h, h*D:(h+1)*D] = 1

    # first-needed DMAs first; constants go on the scalar HWDGE queue
    wq_t = wpool.tile([P, EC, E], BF16, tag="w18")
    xq_t = xpool.tile([P, EC, QB], BF16, tag="xs")
    for ec in range(EC):
        nc.sync.dma_start(wq_t[:, ec, :], wq[ec * P:(ec + 1) * P, :])
        nc.sync.dma_start(xq_t[:, ec, :], xq[ec * P:(ec + 1) * P, :])
    nc.scalar.dma_start(bq_sb[:], bq[:])
    nc.scalar.dma_start(bk_sb[:], bk[:])
    nc.scalar.dma_start(bo_sb[:], bo[:])
    nc.scalar.dma_start(sel[:], seld[:])

    # ones columns for denominator (written once; v-proj copies don't touch col D)
    nc.vector.memset(v_sb[:, :, :, D], 1.0)

    # ---- q^T projection ----
    for mt in range(MT_Q):
        ps = psA.tile([P, 512], F32, tag="psA")
        for ec in range(EC):
            nc.tensor.matmul(ps[:], wq_t[:, ec, mt * P:(mt + 1) * P], xq_t[:, ec, :],
                             start=(ec == 0), stop=(ec == EC - 1))
        nc.vector.tensor_scalar_add(qT[:, mt, :], ps[:], bq_sb[:, mt:mt + 1])

    # ---- k^T projection ----
    wk_t = wpool.tile([P, EC, E], BF16, tag="w18")
    for ec in range(EC):
        nc.sync.dma_start(wk_t[:, ec, :], wk[ec * P:(ec + 1) * P, :])
    for n4 in range(NC4):
        xk_t = xpool.tile([P, EC, 512], BF16, tag="xs")
        nc.sync.dma_start(xk_t[:], xk[:, n4 * 512:(n4 + 1) * 512].rearrange("(ec p) s -> p ec s", p=P))
        for mt in range(MT_Q):
            ps = psA.tile([P, 512], F32, tag="psA")
            for ec in range(EC):
                nc.tensor.matmul(ps[:], wk_t[:, ec, mt * P:(mt + 1) * P], xk_t[:, ec, :],
                                 start=(ec == 0), stop=(ec == EC - 1))
            nc.vector.tensor_scalar_add(kT[:, mt, n4 * 512:(n4 + 1) * 512], ps[:], bk_sb[:, mt:mt + 1])

    # ---- v projection (direct [keys, d]; no bias — folded into bo host-side) ----
    wv_t = wpool.tile([P, EC, E], BF16, tag="w18")
    for ec in range(EC):
        nc.sync.dma_start(wv_t[:, ec, :], wv[ec * P:(ec + 1) * P, :])
    for kt in range(KT):
        xv_t = xvpool.tile([P, EC, P], BF16, tag="xv")
        nc.sync.dma_start(xv_t[:], xv[:, kt * P:(kt + 1) * P].rearrange("(ec p) s -> p ec s", p=P))
        psv = psB.tile([P, E], F32, tag="psB")
        for ec in range(EC):
            nc.tensor.matmul(psv[:, 0:512], xv_t[:, ec, :], wv_t[:, ec, 0:512],
                             start=(ec == 0), stop=(ec == EC - 1))
            nc.tensor.matmul(psv[:, 512:768], xv_t[:, ec, :], wv_t[:, ec, 512:768],
                             start=(ec == 0), stop=(ec == EC - 1))
        # strided copy into per-head slots (leaves ones column intact)
        nc.vector.tensor_copy(v_sb[:, kt, :, 0:D], psv[:].rearrange("p (h d) -> p h d", d=D))

    # ---- attention: head pairs ----
    # Per key tile: both heads' score matmuls are adjacent K=64 ops on
    # disjoint PE row groups (partitions 0-63 / 64-127) -> run concurrently.
    for hp in range(H // 2):
        o_ps = {}
        for i in range(2):
            o_ps[i] = psA.tile([P, 512], F32, tag="psA", name=f"o_ps{i}")
        for kt in range(KT):
            st = psC.tile([P, 2, 512], F32, tag="psC")
            for i in range(2):
                po = D * i      # partition offset of this head's d-rows
                nc.tensor.matmul(st[:, i, :],
                                 kT[po:po + D, hp, kt * P:(kt + 1) * P],
                                 qT[po:po + D, hp, :],
                                 start=True, stop=True)
            ex = epool.tile([P, 2, 512], BF16, tag="ex")
            nc.scalar.activation(ex[:, :, :], st[:, :, :], mybir.ActivationFunctionType.Exp)
            for i in range(2):
                nc.tensor.matmul(o_ps[i][0:D + 1, :],
                                 v_sb[:, kt, 2 * hp + i, :],
                                 ex[:, i, :],
                                 start=(kt == 0), stop=(kt == KT - 1))
        # stage unnormalized outputs (fast PSUM release) and gather denom rows
        for i in range(2):
            nc.vector.tensor_copy(o_raw[:, 2 * hp + i, :], o_ps[i][0:D + 1, :])
            nc.sync.dma_start(dens[2 * hp + i:2 * hp + i + 1, :],
                              o_raw[D:D + 1, 2 * hp + i, :])

    # ---- batched softmax normalization ----
    nc.vector.reciprocal(drec[:], dens[:])
    for hp in range(H // 2):
        for i in range(2):
            h = 2 * hp + i
            po = D * i
            bc_ps = psA.tile([P, 512], F32, tag="psA", name=f"bc{i}")
            nc.tensor.matmul(bc_ps[0:D, :], sel[:, h * D:(h + 1) * D], drec[:],
                             start=True, stop=True)
            bc_sb = spool.tile([D, 512], F32, tag="rb", name=f"bc_sb{i}")
            nc.scalar.copy(bc_sb[:], bc_ps[0:D, :])
            nc.vector.tensor_tensor(o_all[po:po + D, hp, :], o_raw[0:D, h, :], bc_sb[:],
                                    mybir.AluOpType.mult)

    # ---- output projection ----
    wo_t = wpool.tile([P, EC, E], BF16, tag="w18")
    nc.sync.dma_start(wo_t[:], wo[:].rearrange("(ec p) m -> p ec m", p=P))
    ST = QB // P  # 4 s-tiles
    for st4 in range(ST):
        op = psB.tile([P, E], F32, tag="psB")
        for hp in range(H // 2):
            # both heads of the pair contract in one K=128 matmul
            first = (hp == 0)
            last = (hp == H // 2 - 1)
            nc.tensor.matmul(op[:, 0:512],
                             o_all[:, hp, st4 * P:(st4 + 1) * P],
                             wo_t[:, hp, 0:512],
                             start=first, stop=last)
            nc.tensor.matmul(op[:, 512:768],
                             o_all[:, hp, st4 * P:(st4 + 1) * P],
                             wo_t[:, hp, 512:768],
                             start=first, stop=last)
        out_sb = outpool.tile([P, E], F32, tag="outsb")
        nc.vector.tensor_tensor(out_sb[:], op[:], bo_sb[:], mybir.AluOpType.add)
        nc.sync.dma_start(out[st4 * P:(st4 + 1) * P, :], out_sb[:])


_NC_CACHE = None


def _get_nc():
    global _NC_CACHE
    if _NC_CACHE is None:
        _NC_CACHE = build_nc()
    return _NC_CACHE


def make_in_maps(query, key_, value, Wq, bq, Wk, bk, Wv, bv, Wo, bo):
    """Host-side sharding + layout prep. Returns list of 8 input dicts."""
    query = np.asarray(query, dtype=np.float32)
    key_ = np.asarray(key_, dtype=np.float32)
    value = np.asarray(value, dtype=np.float32)
    scale = 1.0 / np.sqrt(np.float32(D))

    import ml_dtypes
    BF = ml_dtypes.bfloat16
    wq_f = (np.ascontiguousarray(np.transpose(np.asarray(Wq, np.float32), (1, 0, 2)).reshape(E, E)) * scale).astype(BF)
    wk_f = np.ascontiguousarray(np.transpose(np.asarray(Wk, np.float32), (1, 0, 2)).reshape(E, E)).astype(BF)
    wv_f = np.ascontiguousarray(np.transpose(np.asarray(Wv, np.float32), (1, 0, 2)).reshape(E, E)).astype(BF)
    wo_f = np.ascontiguousarray(np.asarray(Wo, np.float32)).astype(BF)

    bq_f = (np.asarray(bq, np.float32).reshape(E) * scale).reshape(MT_Q, P).T.copy()
    bk_f = np.asarray(bk, np.float32).reshape(E).reshape(MT_Q, P).T.copy()
    bv_f = np.asarray(bv, np.float32).reshape(E)
    wo_f32 = wo_f.astype(np.float32)
    bo_eff = np.tile((bv_f @ wo_f32 + np.asarray(bo, np.float32)).reshape(1, E), (P, 1)).copy()

    xk_t = [np.ascontiguousarray(key_[b].T).astype(BF) for b in range(B)]
    xv_t = [np.ascontiguousarray(value[b].T).astype(BF) for b in range(B)]

    sel_np = np.zeros((H, H * D), dtype=np.float32)
    for h in range(H):
        sel_np[h, h * D:(h + 1) * D] = 1.0

    in_maps = []
    for core in range(NCORES):
        b = core // (NCORES // B)
        qc = core % (NCORES // B)
        xq_t = np.ascontiguousarray(query[b, qc * QB:(qc + 1) * QB, :].T).astype(BF)
        in_maps.append({
            "xq": xq_t, "xk": xk_t[b], "xv": xv_t[b],
            "wq": wq_f, "wk": wk_f, "wv": wv_f, "wo": wo_f,
            "bq": bq_f, "bk": bk_f, "bo": bo_eff, "seld": sel_np,
        })
    return in_maps


def assemble(results):
    outp = np.empty((B, S, E), dtype=np.float32)
    for core in range(NCORES):
        b = core // (NCORES // B)
        qc = core % (NCORES // B)
        outp[b, qc * QB:(qc + 1) * QB, :] = results[core]["out"]
    return outp


def kernel(query, key_, value, Wq, bq, Wk, bk, Wv, bv, Wo, bo):
    nc = _get_nc()
    in_maps = make_in_maps(query, key_, value, Wq, bq, Wk, bk, Wv, bv, Wo, bo)
    res = run_bass_kernel_spmd(nc, in_maps, core_ids=list(range(NCORES)))
    return assemble(res.results)



# revision 7
# speedup vs baseline: 1.1426x; 1.1426x over previous
# Multi-head attention kernel for Trainium2, sharded over 8 NeuronCores.
#
# Sharding: core = (batch b, query-half qh, head-half hh). Each core handles
# 6 heads (3 head-pairs) x 1024 queries of one batch, computing K/V
# projections only for its own 6 heads (no cross-core recompute). The output
# projection is a PARTIAL sum over the core's 6 heads; the two head-half
# partners' partials are summed on the HOST during assembly (exact fp32 add),
# so no on-chip collective is needed.
#
# Engine balance (per core): ScalarE exp stream = 96 x [128,1024] ~= 128us is
# the kernel floor; PE matmul work ~= 110us fits underneath it. The emission
# order software-pipelines everything against the exp stream:
#   - attention runs as 6 "units" (head-pair x query-512-chunk) x 16 key
#     tiles; scores (2 heads row-packed in PE quadrants) -> exp -> PV with
#     a ones-column producing softmax denominators for free.
#   - all projection work (q/k/v) is emitted as deadline-ordered FILLER
#     chunks interleaved into the attention steps, so the PE chews
#     projections inside the exp windows and the HAM clock never idles.
#   - per-unit softmax normalization: DVE reciprocal at partition 64 of the
#     staged PV output, then a K=1 broadcast matmul from partition 64
#     (selector row of ones) and a DVE multiply. No DMA gathers, no batched
#     tail.
#   - output projection (K=128 per head pair, accumulated over 3 pairs) runs
#     in a short tail reusing the score-PSUM slots.
# PSUM budget: scores 2x[128,2,512] (4 banks) + PV accumulators 2x[65,512]
# (2 banks) + projection scratch 2x[128,512] (2 banks) = 8 banks exactly.

import numpy as np
import os
from contextlib import ExitStack

_DEBUG = os.environ.get('KDEBUG', '0') == '1'

import concourse.bass as bass
import concourse.mybir as mybir
import concourse.tile as tile
from concourse import bacc
from concourse.bass_utils import run_bass_kernel_spmd

F32 = mybir.dt.float32
BF16 = mybir.dt.bfloat16
P = 128
E = 768
S = 2048
B = 2
H = 12
D = 64
NCORES = 8
EC = E // P        # 6 e-chunks (contraction over hidden)
KT = S // P        # 16 key tiles
HL = 6             # heads per core
MT = HL * D // P   # 3 m-tiles (head pairs) per core
QB = 1024          # queries per core
QC = 2             # query 512-chunks per core
NC4 = S // 512     # 4 n-slices of k^T


def build_nc():
    nc = bacc.Bacc("TRN2", debug=False)

    # DRAM I/O (per-core shapes; same NEFF on all 8 cores)
    xq = nc.dram_tensor("xq", (E, QB), BF16, kind="ExternalInput")    # query[b, qh].T
    xk = nc.dram_tensor("xk", (E, S), BF16, kind="ExternalInput")     # key[b].T
    xv = nc.dram_tensor("xv", (E, S), BF16, kind="ExternalInput")     # value[b].T
    wq = nc.dram_tensor("wq", (E, HL * D), BF16, kind="ExternalInput")  # pre-scaled 1/sqrt(D)
    wk = nc.dram_tensor("wk", (E, HL * D), BF16, kind="ExternalInput")
    wv = nc.dram_tensor("wv", (E, HL * D), BF16, kind="ExternalInput")
    wo = nc.dram_tensor("wo", (HL * D, E), BF16, kind="ExternalInput")
    bq = nc.dram_tensor("bq", (P, MT), F32, kind="ExternalInput")     # per-partition bias per m-tile
    bk = nc.dram_tensor("bk", (P, MT), F32, kind="ExternalInput")
    bo = nc.dram_tensor("bo", (P, E), F32, kind="ExternalInput")      # partial (bv@Wo [+ bo]), broadcast
    out = nc.dram_tensor("out", (QB, E), F32, kind="ExternalOutput")  # PARTIAL over this core's heads
    dbg = None
    if _DEBUG:
        dbg = nc.dram_tensor("dbg", (D + 1, HL * QC * 512), F32, kind="ExternalOutput")

    with tile.TileContext(nc) as tc:
        with ExitStack() as ctx:
            _emit(ctx, tc, nc, xq, xk, xv, wq, wk, wv, wo, bq, bk, bo, out, dbg)
    nc.compile()
    return nc


def _emit(ctx, tc, nc, xq, xk, xv, wq, wk, wv, wo, bq, bk, bo, out, dbg=None):
    # ---- pools ----
    persist = ctx.enter_context(tc.tile_pool(name="persist", bufs=1))
    wpool = ctx.enter_context(tc.tile_pool(name="wpool", bufs=3))
    xvpool = ctx.enter_context(tc.tile_pool(name="xvpool", bufs=3))
    epool = ctx.enter_context(tc.tile_pool(name="epool", bufs=6))
    outpool = ctx.enter_context(tc.tile_pool(name="outpool", bufs=2))
    # PSUM pools: 4 + 2 + 2 = 8 banks
    psC = ctx.enter_context(tc.tile_pool(name="psC", bufs=2, space="PSUM"))    # scores [128,2,512]
    opool = ctx.enter_context(tc.tile_pool(name="opool", bufs=2, space="PSUM"))  # PV accum [65,512]
    pj = ctx.enter_context(tc.tile_pool(name="pj", bufs=2, space="PSUM"))      # proj scratch [128,512]

    # ---- persistent SBUF tensors ----
    qT = persist.tile([P, MT, QB], BF16)          # q^T [384, 1024]
    kT = persist.tile([P, MT, S], BF16)           # k^T [384, 2048]
    v_sb = persist.tile([P, KT, HL, D + 1], BF16)  # v + ones column per head
    o_all = persist.tile([P, MT, QB], BF16)       # normalized o^T, pairs in partition halves
    o_raw = persist.tile([D + 1, HL, QC, 512], F32)  # staged unnormalized o^T + denom row
    drec = persist.tile([D + 1, HL, QC, 512], F32)   # reciprocal denominators (row 64 used)
    sel64 = persist.tile([D + 1, D], F32)         # row 64 = ones (K=1 broadcast selector)
    bq_sb = persist.tile([P, MT], F32)
    bk_sb = persist.tile([P, MT], F32)
    bo_sb = persist.tile([P, E], F32)
    xq_t = persist.tile([P, EC, QC, 512], BF16)   # resident query slices
    xk_t = persist.tile([P, EC, NC4, 512], BF16)  # resident key slices

    # ---- DMAs: first-needed first; weights/biases on scalar queue ----
    wq_t = wpool.tile([P, EC, HL * D], BF16, tag="w")
    nc.scalar.dma_start(wq_t[:], wq[:].rearrange("(ec p) m -> p ec m", p=P))
    nc.sync.dma_start(xq_t[:, :, 0, :], xq[:, 0:512].rearrange("(ec p) s -> p ec s", p=P))
    nc.sync.dma_start(xk_t[:, :, 0, :], xk[:, 0:512].rearrange("(ec p) s -> p ec s", p=P))
    wk_t = wpool.tile([P, EC, HL * D], BF16, tag="w")
    nc.scalar.dma_start(wk_t[:], wk[:].rearrange("(ec p) m -> p ec m", p=P))
    wv_t = wpool.tile([P, EC, HL * D], BF16, tag="w")
    nc.scalar.dma_start(wv_t[:], wv[:].rearrange("(ec p) m -> p ec m", p=P))
    nc.sync.dma_start(xq_t[:, :, 1, :], xq[:, 512:1024].rearrange("(ec p) s -> p ec s", p=P))
    for n4 in range(1, NC4):
        nc.sync.dma_start(xk_t[:, :, n4, :],
                          xk[:, n4 * 512:(n4 + 1) * 512].rearrange("(ec p) s -> p ec s", p=P))
    nc.scalar.dma_start(bq_sb[:], bq[:])
    nc.scalar.dma_start(bk_sb[:], bk[:])
    nc.scalar.dma_start(bo_sb[:], bo[:])

    # constants: ones column for denominators, selector row for broadcast
    nc.vector.memset(v_sb[:, :, :, D], 1.0)
    nc.vector.memset(sel64[D:D + 1, :], 1.0)

    # ---- projection emitters (interleaved as filler work) ----
    def emit_q(mt, qc):
        ps = pj.tile([P, 512], F32, tag="pj")
        for ec in range(EC):
            nc.tensor.matmul(ps[:], wq_t[:, ec, mt * P:(mt + 1) * P], xq_t[:, ec, qc, :],
                             start=(ec == 0), stop=(ec == EC - 1))
        nc.vector.tensor_scalar_add(qT[:, mt, qc * 512:(qc + 1) * 512], ps[:], bq_sb[:, mt:mt + 1])

    def emit_k(mt, n4):
        ps = pj.tile([P, 512], F32, tag="pj")
        for ec in range(EC):
            nc.tensor.matmul(ps[:], wk_t[:, ec, mt * P:(mt + 1) * P], xk_t[:, ec, n4, :],
                             start=(ec == 0), stop=(ec == EC - 1))
        nc.vector.tensor_scalar_add(kT[:, mt, n4 * 512:(n4 + 1) * 512], ps[:], bk_sb[:, mt:mt + 1])

    def emit_v(kt):
        xv_t = xvpool.tile([P, EC, P], BF16, tag="xv")
        nc.gpsimd.dma_start(xv_t[:], xv[:, kt * P:(kt + 1) * P].rearrange("(ec p) s -> p ec s", p=P))
        ps = pj.tile([P, 512], F32, tag="pj")  # only 384 used
        for ec in range(EC):
            nc.tensor.matmul(ps[:, 0:HL * D], xv_t[:, ec, :], wv_t[:, ec, :],
                             start=(ec == 0), stop=(ec == EC - 1))
        nc.vector.tensor_copy(v_sb[:, kt, :, 0:D],
                              ps[:, 0:HL * D].rearrange("p (h d) -> p h d", d=D))

    # Deadline-ordered filler queue: (deadline_step, emit_fn). Steps run
    # 0..95 (6 units x 16 key tiles). Forced at deadline; otherwise drained
    # at DRAIN_BUDGET items/step to spread PE work under the exp stream.
    fillers = []
    for n4 in range(1, NC4):
        fillers.append((3 * n4 - 1, lambda n4=n4: emit_k(0, n4)))     # unit0 kt=4n4 needs slice n4
    for kt in range(1, KT):
        # MUST be emitted before PV(unit0, kt) reads v_sb[kt] — Tile deps
        # follow emission order, so a late v-write would be ordered AFTER
        # the stale read. Deadline kt = drained at top of step kt.
        fillers.append((kt, lambda kt=kt: emit_v(kt)))
    fillers.append((12, lambda: emit_q(0, 1)))                        # unit1 = (pair0, qc1)
    for n4 in range(NC4):
        fillers.append((26 + 2 * n4, lambda n4=n4: emit_k(1, n4)))    # unit2 = (pair1, qc0)
    fillers.append((28, lambda: emit_q(1, 0)))
    fillers.append((44, lambda: emit_q(1, 1)))
    for n4 in range(NC4):
        fillers.append((58 + 2 * n4, lambda n4=n4: emit_k(2, n4)))    # unit4 = (pair2, qc0)
    fillers.append((60, lambda: emit_q(2, 0)))
    fillers.append((76, lambda: emit_q(2, 1)))
    fillers.sort(key=lambda t: t[0])
    fidx = [0]

    def drain_fillers(step, budget):
        n = 0
        while fidx[0] < len(fillers) and (fillers[fidx[0]][0] <= step or n < budget):
            fillers[fidx[0]][1]()
            fidx[0] += 1
            n += 1

    # ---- prologue: minimal work before the exp stream starts ----
    emit_q(0, 0)
    emit_k(0, 0)
    emit_v(0)

    # ---- attention units: (head pair, query chunk), 16 key tiles each ----
    for u in range(MT * QC):
        pair, qc = u // QC, u % QC
        o_ps = {i: opool.tile([D + 1, 512], F32, tag="o", name=f"o{i}") for i in range(2)}
        for kt in range(KT):
            drain_fillers(u * KT + kt, 2)
            st = psC.tile([P, 2, 512], F32, tag="sc")
            for i in range(2):
                po = D * i
                nc.tensor.matmul(st[:, i, :],
                                 kT[po:po + D, pair, kt * P:(kt + 1) * P],
                                 qT[po:po + D, pair, qc * 512:(qc + 1) * 512],
                                 start=True, stop=True)
            ex = epool.tile([P, 2, 512], BF16, tag="ex")
            nc.scalar.activation(ex[:, :, :], st[:, :, :], mybir.ActivationFunctionType.Exp)
            for i in range(2):
                nc.tensor.matmul(o_ps[i][:, :],
                                 v_sb[:, kt, 2 * pair + i, :],
                                 ex[:, i, :],
                                 start=(kt == 0), stop=(kt == KT - 1))
        # ---- per-unit softmax normalization ----
        for i in range(2):
            h = 2 * pair + i
            nc.vector.tensor_copy(o_raw[:, h, qc, :], o_ps[i][:, :])  # stage; frees PSUM fast
        nc.vector.reciprocal(drec[D:D + 1, 2 * pair:2 * pair + 2, qc, :],
                             o_raw[D:D + 1, 2 * pair:2 * pair + 2, qc, :])
        for i in range(2):
            h = 2 * pair + i
            bc = pj.tile([D, 512], F32, tag="pj", name=f"bc{i}")
            nc.tensor.matmul(bc[:], sel64[D:D + 1, :], drec[D:D + 1, h, qc, :],
                             start=True, stop=True)
            nc.vector.tensor_tensor(o_all[i * D:(i + 1) * D, pair, qc * 512:(qc + 1) * 512],
                                    o_raw[0:D, h, qc, :], bc[:], mybir.AluOpType.mult)

    if dbg is not None:
        nc.sync.dma_start(dbg[:, :], o_raw[:].rearrange("p a b c -> p (a b c)"))

    # ---- output projection (partial over this core's 6 heads) ----
    wo_t = wpool.tile([P, MT, E], BF16, tag="w")
    nc.scalar.dma_start(wo_t[:], wo[:].rearrange("(mt p) e -> p mt e", p=P))
    ST = QB // P  # 8 query tiles
    for st8 in range(ST):
        op = psC.tile([P, 2, 512], F32, tag="sc", name="oproj")
        for pair in range(MT):
            first, last = (pair == 0), (pair == MT - 1)
            nc.tensor.matmul(op[:, 0, :], o_all[:, pair, st8 * P:(st8 + 1) * P],
                             wo_t[:, pair, 0:512], start=first, stop=last)
            nc.tensor.matmul(op[:, 1, 0:256], o_all[:, pair, st8 * P:(st8 + 1) * P],
                             wo_t[:, pair, 512:768], start=first, stop=last)
        out_sb = outpool.tile([P, E], F32, tag="osb")
        nc.vector.tensor_tensor(out_sb[:, 0:512], op[:, 0, :], bo_sb[:, 0:512],
                                mybir.AluOpType.add)
        nc.vector.tensor_tensor(out_sb[:, 512:768], op[:, 1, 0:256], bo_sb[:, 512:768],
                                mybir.AluOpType.add)
        nc.sync.dma_start(out[st8 * P:(st8 + 1) * P, :], out_sb[:])


_NC_CACHE = None


def _get_nc():
    global _NC_CACHE
    if _NC_CACHE is None:
        _NC_CACHE = build_nc()
    return _NC_CACHE


def make_in_maps(query, key_, value, Wq, bq, Wk, bk, Wv, bv, Wo, bo):
    """Host-side sharding + layout prep. Returns list of 8 input dicts."""
    import ml_dtypes
    BF = ml_dtypes.bfloat16
    query = np.asarray(query, dtype=np.float32)
    key_ = np.asarray(key_, dtype=np.float32)
    value = np.asarray(value, dtype=np.float32)
    scale = np.float32(1.0 / np.sqrt(np.float32(D)))

    Wq = np.asarray(Wq, np.float32)
    Wk = np.asarray(Wk, np.float32)
    Wv = np.asarray(Wv, np.float32)
    Wo = np.asarray(Wo, np.float32)
    bq_f = np.asarray(bq, np.float32)
    bk_f = np.asarray(bk, np.float32)
    bv_f = np.asarray(bv, np.float32)
    bo_f = np.asarray(bo, np.float32)

    xk_t = [np.ascontiguousarray(key_[b].T).astype(BF) for b in range(B)]
    xv_t = [np.ascontiguousarray(value[b].T).astype(BF) for b in range(B)]
    xq_t = {}
    for b in range(B):
        for qh in range(2):
            xq_t[(b, qh)] = np.ascontiguousarray(
                query[b, qh * QB:(qh + 1) * QB, :].T).astype(BF)

    per_hh = {}
    for hh in range(2):
        hs = slice(hh * HL, (hh + 1) * HL)
        wq_f = (np.transpose(Wq[hs], (1, 0, 2)).reshape(E, HL * D) * scale).astype(BF)
        wk_f = np.transpose(Wk[hs], (1, 0, 2)).reshape(E, HL * D).astype(BF)
        wv_f = np.transpose(Wv[hs], (1, 0, 2)).reshape(E, HL * D).astype(BF)
        wo_f = np.ascontiguousarray(Wo[hh * HL * D:(hh + 1) * HL * D, :]).astype(BF)
        bq_p = (bq_f[hs].reshape(HL * D) * scale).reshape(MT, P).T.copy()
        bk_p = bk_f[hs].reshape(HL * D).reshape(MT, P).T.copy()
        # v-bias folded through this core's Wo rows; bo itself only on hh=0
        bo_eff = bv_f[hs].reshape(HL * D) @ wo_f.astype(np.float32)
        if hh == 0:
            bo_eff = bo_eff + bo_f
        per_hh[hh] = dict(
            wq=wq_f, wk=wk_f, wv=wv_f, wo=wo_f, bq=bq_p, bk=bk_p,
            bo=np.tile(bo_eff.reshape(1, E), (P, 1)).copy(),
        )

    in_maps = []
    for core in range(NCORES):
        b, qh, hh = core // 4, (core // 2) % 2, core % 2
        m = dict(per_hh[hh])
        m["xq"] = xq_t[(b, qh)]
        m["xk"] = xk_t[b]
        m["xv"] = xv_t[b]
        in_maps.append(m)
    return in_maps


def assemble(results):
    outp = np.empty((B, S, E), dtype=np.float32)
    for b in range(B):
        for qh in range(2):
            c0 = b * 4 + qh * 2
            outp[b, qh * QB:(qh + 1) * QB, :] = results[c0]["out"] + results[c0 + 1]["out"]
    return outp


def kernel(query, key_, value, Wq, bq, Wk, bk, Wv, bv, Wo, bo):
    nc = _get_nc()
    in_maps = make_in_maps(query, key_, value, Wq, bq, Wk, bk, Wv, bv, Wo, bo)
    res = run_bass_kernel_spmd(nc, in_maps, core_ids=list(range(NCORES)))
    return assemble(res.results)


# revision 11
# speedup vs baseline: 1.2289x; 1.0755x over previous
# Multi-head attention kernel for Trainium2, sharded over 8 NeuronCores.
#
# Sharding: core = (batch b, query-half qh, head-half hh). Each core handles
# 6 heads (3 head-pairs) x 1024 queries of one batch, computing K/V
# projections only for its own 6 heads (no cross-core recompute). The output
# projection is a PARTIAL sum over the core's 6 heads; the two head-half
# partners' partials are summed on the HOST during assembly (exact fp32 add),
# so no on-chip collective is needed.
#
# Engine balance (per core): ScalarE exp stream = 96 x [128,1024] ~= 128us is
# the kernel floor; PE matmul work ~= 110us fits underneath it. The emission
# order software-pipelines everything against the exp stream:
#   - attention runs as 6 "units" (head-pair x query-512-chunk) x 16 key
#     tiles; scores (2 heads row-packed in PE quadrants) -> exp -> PV with
#     a ones-column producing softmax denominators for free.
#   - all projection work (q/k/v) is emitted as deadline-ordered FILLER
#     chunks interleaved into the attention steps, so the PE chews
#     projections inside the exp windows and the HAM clock never idles.
#   - per-unit softmax normalization: DVE reciprocal at partition 64 of the
#     staged PV output, then a K=1 broadcast matmul from partition 64
#     (selector row of ones) and a DVE multiply. No DMA gathers, no batched
#     tail.
#   - output projection (K=128 per head pair, accumulated over 3 pairs) runs
#     in a short tail reusing the score-PSUM slots.
# PSUM budget: scores 2x[128,2,512] (4 banks) + PV accumulators 2x[65,512]
# (2 banks) + projection scratch 2x[128,512] (2 banks) = 8 banks exactly.

import numpy as np
import os
from contextlib import ExitStack

_DEBUG = os.environ.get('KDEBUG', '0') == '1'

import concourse.bass as bass
import concourse.mybir as mybir
import concourse.tile as tile
from concourse import bacc
from concourse.bass_utils import run_bass_kernel_spmd

F32 = mybir.dt.float32
BF16 = mybir.dt.bfloat16
P = 128
E = 768
S = 2048
B = 2
H = 12
D = 64
NCORES = 8
EC = E // P        # 6 e-chunks (contraction over hidden)
KT = S // P        # 16 key tiles
HL = 6             # heads per core
MT = HL * D // P   # 3 m-tiles (head pairs) per core
QB = 1024          # queries per core
QC = 2             # query 512-chunks per core
NC4 = S // 512     # 4 n-slices of k^T


def build_nc():
    nc = bacc.Bacc("TRN2", debug=False)

    # DRAM I/O (per-core shapes; same NEFF on all 8 cores)
    # all inputs pre-arranged on host into on-chip [128-partition, ...] layout
    # so every DMA is a contiguous full-bandwidth copy
    xq = nc.dram_tensor("xq", (QC, P, EC * 512), BF16, kind="ExternalInput")
    xk = nc.dram_tensor("xk", (NC4, P, EC * 512), BF16, kind="ExternalInput")
    xv = nc.dram_tensor("xv", (KT, P, EC * P), BF16, kind="ExternalInput")
    wq = nc.dram_tensor("wq", (MT, P, EC * P), BF16, kind="ExternalInput")  # pre-scaled 1/sqrt(D)
    wk = nc.dram_tensor("wk", (MT, P, EC * P), BF16, kind="ExternalInput")
    wv = nc.dram_tensor("wv", (P, EC * HL * D), BF16, kind="ExternalInput")
    wo = nc.dram_tensor("wo", (P, MT * E), BF16, kind="ExternalInput")
    bq = nc.dram_tensor("bq", (P, MT), F32, kind="ExternalInput")     # per-partition bias per m-tile
    bk = nc.dram_tensor("bk", (P, MT), F32, kind="ExternalInput")
    bo = nc.dram_tensor("bo", (P, E), F32, kind="ExternalInput")      # partial (bv@Wo [+ bo]), broadcast
    out = nc.dram_tensor("out", (QB, E), F32, kind="ExternalOutput")  # PARTIAL over this core's heads
    dbg = None
    if _DEBUG:
        dbg = nc.dram_tensor("dbg", (D + 1, HL * QC * 512), F32, kind="ExternalOutput")

    with tile.TileContext(nc) as tc:
        with ExitStack() as ctx:
            _emit(ctx, tc, nc, xq, xk, xv, wq, wk, wv, wo, bq, bk, bo, out, dbg)
    nc.compile()
    return nc


def _emit(ctx, tc, nc, xq, xk, xv, wq, wk, wv, wo, bq, bk, bo, out, dbg=None):
    # ---- pools ----
    persist = ctx.enter_context(tc.tile_pool(name="persist", bufs=1))
    wpool = ctx.enter_context(tc.tile_pool(name="wpool", bufs=3))
    xvpool = ctx.enter_context(tc.tile_pool(name="xvpool", bufs=3))
    epool = ctx.enter_context(tc.tile_pool(name="epool", bufs=6))
    outpool = ctx.enter_context(tc.tile_pool(name="outpool", bufs=2))
    # PSUM pools: 4 + 2 + 2 = 8 banks
    psC = ctx.enter_context(tc.tile_pool(name="psC", bufs=2, space="PSUM"))    # scores [128,2,512]
    opool = ctx.enter_context(tc.tile_pool(name="opool", bufs=2, space="PSUM"))  # PV accum [65,512]
    pj = ctx.enter_context(tc.tile_pool(name="pj", bufs=2, space="PSUM"))      # proj scratch [128,512]

    # ---- persistent SBUF tensors ----
    qT = persist.tile([P, MT, QB], BF16)          # q^T [384, 1024]
    kT = persist.tile([P, MT, S], BF16)           # k^T [384, 2048]
    v_sb = persist.tile([P, KT, HL, D + 1], BF16)  # v + ones column per head
    o_all = persist.tile([P, MT, QB], BF16)       # normalized o^T, pairs in partition halves
    o_raw = persist.tile([D + 1, HL, QC, 512], F32)  # staged unnormalized o^T + denom row
    drecf = persist.tile([D + 1, 2, 512], F32)    # per-unit reciprocal scratch (row 64)
    drecb = persist.tile([D + 1, 2, 512], BF16)   # bf16 copy feeding the bc matmul
    sel64 = persist.tile([D + 1, D], BF16)        # row 64 = ones (K=1 broadcast selector)
    bq_sb = persist.tile([P, MT], F32)
    bk_sb = persist.tile([P, MT], F32)
    bo_sb = persist.tile([P, E], F32)
    xq_t = persist.tile([P, QC, EC, 512], BF16)   # resident query slices
    xk_t = persist.tile([P, NC4, EC, 512], BF16)  # resident key slices
    wq_t = persist.tile([P, MT, EC, P], BF16)
    wk_t = persist.tile([P, MT, EC, P], BF16)

    # ---- DMAs: first-needed first; weights on scalar queue, x on sync ----
    nc.scalar.dma_start(wq_t[:, 0, :, :].rearrange("p a b -> p (a b)"), wq[0])
    nc.scalar.dma_start(wk_t[:, 0, :, :].rearrange("p a b -> p (a b)"), wk[0])
    nc.sync.dma_start(bq_sb[:], bq[:])
    nc.sync.dma_start(bk_sb[:], bk[:])
    nc.sync.dma_start(xq_t[:, 0, :, :].rearrange("p a b -> p (a b)"), xq[0])
    nc.sync.dma_start(xk_t[:, 0, :, :].rearrange("p a b -> p (a b)"), xk[0])
    wv_t = wpool.tile([P, EC, HL * D], BF16, tag="w")
    nc.scalar.dma_start(wv_t[:].rearrange("p a b -> p (a b)"), wv[:])
    for mt in range(1, MT):
        nc.scalar.dma_start(wq_t[:, mt, :, :].rearrange("p a b -> p (a b)"), wq[mt])
        nc.scalar.dma_start(wk_t[:, mt, :, :].rearrange("p a b -> p (a b)"), wk[mt])
    nc.sync.dma_start(xk_t[:, 1, :, :].rearrange("p a b -> p (a b)"), xk[1])
    nc.sync.dma_start(xq_t[:, 1, :, :].rearrange("p a b -> p (a b)"), xq[1])
    nc.sync.dma_start(xk_t[:, 2, :, :].rearrange("p a b -> p (a b)"), xk[2])
    nc.sync.dma_start(xk_t[:, 3, :, :].rearrange("p a b -> p (a b)"), xk[3])
    nc.sync.dma_start(bo_sb[:], bo[:])

    # constants: ones column for denominators, selector row for broadcast
    nc.vector.memset(v_sb[:, :, :, D], 1.0)
    nc.vector.memset(sel64[D:D + 1, :], 1.0)

    # ---- projection emitters (interleaved as filler work) ----
    def emit_q(mt, qc):
        ps = pj.tile([P, 512], F32, tag="pj")
        for ec in range(EC):
            nc.tensor.matmul(ps[:], wq_t[:, mt, ec, :], xq_t[:, qc, ec, :],
                             start=(ec == 0), stop=(ec == EC - 1))
        nc.vector.tensor_scalar_add(qT[:, mt, qc * 512:(qc + 1) * 512], ps[:], bq_sb[:, mt:mt + 1])

    def emit_k(mt, n4):
        ps = pj.tile([P, 512], F32, tag="pj")
        for ec in range(EC):
            nc.tensor.matmul(ps[:], wk_t[:, mt, ec, :], xk_t[:, n4, ec, :],
                             start=(ec == 0), stop=(ec == EC - 1))
        nc.vector.tensor_scalar_add(kT[:, mt, n4 * 512:(n4 + 1) * 512], ps[:], bk_sb[:, mt:mt + 1])

    def emit_v(kt):
        xv_t = xvpool.tile([P, EC, P], BF16, tag="xv")
        nc.gpsimd.dma_start(xv_t[:].rearrange("p a b -> p (a b)"), xv[kt])
        ps = pj.tile([P, 512], F32, tag="pj")  # only 384 used
        for ec in range(EC):
            nc.tensor.matmul(ps[:, 0:HL * D], xv_t[:, ec, :], wv_t[:, ec, :],
                             start=(ec == 0), stop=(ec == EC - 1))
        nc.vector.tensor_copy(v_sb[:, kt, :, 0:D],
                              ps[:, 0:HL * D].rearrange("p (h d) -> p h d", d=D))

    # Deadline-ordered filler queue: (deadline_step, emit_fn). Steps run
    # 0..95 (6 units x 16 key tiles). Forced at deadline; otherwise drained
    # at DRAIN_BUDGET items/step to spread PE work under the exp stream.
    fillers = []
    for n4 in range(1, NC4):
        fillers.append((3 * n4 - 1, lambda n4=n4: emit_k(0, n4)))     # unit0 kt=4n4 needs slice n4
    for kt in range(1, KT):
        # MUST be emitted before PV(unit0, kt) reads v_sb[kt] — Tile deps
        # follow emission order, so a late v-write would be ordered AFTER
        # the stale read. Deadline kt = drained at top of step kt.
        fillers.append((kt, lambda kt=kt: emit_v(kt)))
    fillers.append((12, lambda: emit_q(0, 1)))                        # unit1 = (pair0, qc1)
    for n4 in range(NC4):
        fillers.append((26 + 2 * n4, lambda n4=n4: emit_k(1, n4)))    # unit2 = (pair1, qc0)
    fillers.append((28, lambda: emit_q(1, 0)))
    fillers.append((44, lambda: emit_q(1, 1)))
    for n4 in range(NC4):
        fillers.append((58 + 2 * n4, lambda n4=n4: emit_k(2, n4)))    # unit4 = (pair2, qc0)
    fillers.append((60, lambda: emit_q(2, 0)))
    fillers.append((76, lambda: emit_q(2, 1)))
    fillers.sort(key=lambda t: t[0])
    fidx = [0]

    def drain_fillers(step, budget):
        n = 0
        while fidx[0] < len(fillers) and (fillers[fidx[0]][0] <= step or n < budget):
            fillers[fidx[0]][1]()
            fidx[0] += 1
            n += 1

    # ---- prologue: minimal work before the exp stream starts ----
    emit_q(0, 0)
    emit_k(0, 0)
    emit_v(0)

    # ---- attention units: (head pair, query chunk), 16 key tiles each ----
    # norm(u) is EMITTED a few steps into unit u+1 so its reciprocal->bc
    # chain never head-of-line-blocks the PE between units.
    def make_norm(pair, qc, o_ps):
        def norm():
            for i in range(2):
                h = 2 * pair + i
                nc.vector.tensor_copy(o_raw[:, h, qc, :], o_ps[i][:, :])  # stage; frees PSUM
            nc.vector.reciprocal(drecf[D:D + 1, :, :],
                                 o_raw[D:D + 1, 2 * pair:2 * pair + 2, qc, :])
            nc.vector.tensor_copy(drecb[D:D + 1, :, :], drecf[D:D + 1, :, :])
            for i in range(2):
                h = 2 * pair + i
                bc = pj.tile([D, 512], F32, tag="pj", name=f"bc{i}")
                nc.tensor.matmul(bc[:], sel64[D:D + 1, :], drecb[D:D + 1, i, :],
                                 start=True, stop=True)
                nc.vector.tensor_tensor(o_all[i * D:(i + 1) * D, pair, qc * 512:(qc + 1) * 512],
                                        o_raw[0:D, h, qc, :], bc[:], mybir.AluOpType.mult)
        return norm

    pending_norm = None
    for u in range(MT * QC):
        pair, qc = u // QC, u % QC
        o_ps = {i: opool.tile([D + 1, 512], F32, tag="o", name=f"o{i}") for i in range(2)}
        for kt in range(KT):
            drain_fillers(u * KT + kt, 2)
            if kt == 5 and pending_norm is not None:
                pending_norm()
                pending_norm = None
            st = psC.tile([P, 2, 512], F32, tag="sc")
            for i in range(2):
                po = D * i
                nc.tensor.matmul(st[:, i, :],
                                 kT[po:po + D, pair, kt * P:(kt + 1) * P],
                                 qT[po:po + D, pair, qc * 512:(qc + 1) * 512],
                                 start=True, stop=True)
            ex = epool.tile([P, 2, 512], BF16, tag="ex")
            nc.scalar.activation(ex[:, :, :], st[:, :, :], mybir.ActivationFunctionType.Exp)
            for i in range(2):
                nc.tensor.matmul(o_ps[i][:, :],
                                 v_sb[:, kt, 2 * pair + i, :],
                                 ex[:, i, :],
                                 start=(kt == 0), stop=(kt == KT - 1))
        pending_norm = make_norm(pair, qc, o_ps)
    pending_norm()

    if dbg is not None:
        nc.sync.dma_start(dbg[:, :], o_raw[:].rearrange("p a b c -> p (a b c)"))

    # ---- output projection (partial over this core's 6 heads) ----
    wo_t = wpool.tile([P, MT, E], BF16, tag="w")
    nc.scalar.dma_start(wo_t[:].rearrange("p a b -> p (a b)"), wo[:])
    ST = QB // P  # 8 query tiles
    for st8 in range(ST):
        op = psC.tile([P, 2, 512], F32, tag="sc", name="oproj")
        for pair in range(MT):
            first, last = (pair == 0), (pair == MT - 1)
            nc.tensor.matmul(op[:, 0, :], o_all[:, pair, st8 * P:(st8 + 1) * P],
                             wo_t[:, pair, 0:512], start=first, stop=last)
            nc.tensor.matmul(op[:, 1, 0:256], o_all[:, pair, st8 * P:(st8 + 1) * P],
                             wo_t[:, pair, 512:768], start=first, stop=last)
        out_sb = outpool.tile([P, E], F32, tag="osb")
        nc.vector.tensor_tensor(out_sb[:, 0:512], op[:, 0, :], bo_sb[:, 0:512],
                                mybir.AluOpType.add)
        nc.vector.tensor_tensor(out_sb[:, 512:768], op[:, 1, 0:256], bo_sb[:, 512:768],
                                mybir.AluOpType.add)
        nc.sync.dma_start(out[st8 * P:(st8 + 1) * P, :], out_sb[:])


_NC_CACHE = None


def _get_nc():
    global _NC_CACHE
    if _NC_CACHE is None:
        _NC_CACHE = build_nc()
    return _NC_CACHE


def make_in_maps(query, key_, value, Wq, bq, Wk, bk, Wv, bv, Wo, bo):
    """Host-side sharding + layout prep. Returns list of 8 input dicts."""
    import ml_dtypes
    BF = ml_dtypes.bfloat16
    query = np.asarray(query, dtype=np.float32)
    key_ = np.asarray(key_, dtype=np.float32)
    value = np.asarray(value, dtype=np.float32)
    scale = np.float32(1.0 / np.sqrt(np.float32(D)))

    Wq = np.asarray(Wq, np.float32)
    Wk = np.asarray(Wk, np.float32)
    Wv = np.asarray(Wv, np.float32)
    Wo = np.asarray(Wo, np.float32)
    bq_f = np.asarray(bq, np.float32)
    bk_f = np.asarray(bk, np.float32)
    bv_f = np.asarray(bv, np.float32)
    bo_f = np.asarray(bo, np.float32)

    def pem(a):
        # [E, M] -> [128p, EC, M] -> flat [128, EC*M]
        E_, m = a.shape
        return np.ascontiguousarray(a.reshape(EC, P, m).transpose(1, 0, 2).reshape(P, EC * m))

    def xslices(a, width):
        # [E, S] -> [S//width, 128, EC*width]
        E_, s = a.shape
        n = s // width
        r = a.reshape(EC, P, n, width).transpose(2, 1, 0, 3)
        return np.ascontiguousarray(r.reshape(n, P, EC * width))

    xk_t = [xslices(key_[b].T, 512).astype(BF) for b in range(B)]
    xv_t = [xslices(value[b].T, P).astype(BF) for b in range(B)]
    xq_t = {}
    for b in range(B):
        for qh in range(2):
            xq_t[(b, qh)] = xslices(query[b, qh * QB:(qh + 1) * QB, :].T, 512).astype(BF)

    per_hh = {}
    for hh in range(2):
        hs = slice(hh * HL, (hh + 1) * HL)
        wq_f = np.transpose(Wq[hs], (1, 0, 2)).reshape(E, HL * D) * scale
        wk_f = np.transpose(Wk[hs], (1, 0, 2)).reshape(E, HL * D)
        wv_f = np.transpose(Wv[hs], (1, 0, 2)).reshape(E, HL * D)
        wo_f = Wo[hh * HL * D:(hh + 1) * HL * D, :]
        # wq/wk: per-m-tile chunks [MT, 128, EC*128]
        wq_c = np.stack([pem(wq_f[:, mt * P:(mt + 1) * P]) for mt in range(MT)]).astype(BF)
        wk_c = np.stack([pem(wk_f[:, mt * P:(mt + 1) * P]) for mt in range(MT)]).astype(BF)
        wv_c = pem(wv_f).astype(BF)
        # wo: [384, 768] -> [128, MT*768], partition p holds row mt*128+p
        wo_c = np.ascontiguousarray(
            wo_f.reshape(MT, P, E).transpose(1, 0, 2).reshape(P, MT * E)).astype(BF)
        bq_p = (bq_f[hs].reshape(HL * D) * scale).reshape(MT, P).T.copy()
        bk_p = bk_f[hs].reshape(HL * D).reshape(MT, P).T.copy()
        # v-bias folded through this core's Wo rows; bo itself only on hh=0
        bo_eff = bv_f[hs].reshape(HL * D) @ wo_f
        if hh == 0:
            bo_eff = bo_eff + bo_f
        per_hh[hh] = dict(
            wq=wq_c, wk=wk_c, wv=wv_c, wo=wo_c, bq=bq_p, bk=bk_p,
            bo=np.tile(bo_eff.reshape(1, E), (P, 1)).astype(np.float32).copy(),
        )

    in_maps = []
    for core in range(NCORES):
        b, qh, hh = core // 4, (core // 2) % 2, core % 2
        m = dict(per_hh[hh])
        m["xq"] = xq_t[(b, qh)]
        m["xk"] = xk_t[b]
        m["xv"] = xv_t[b]
        in_maps.append(m)
    return in_maps


def assemble(results):
    outp = np.empty((B, S, E), dtype=np.float32)
    for b in range(B):
        for qh in range(2):
            c0 = b * 4 + qh * 2
            outp[b, qh * QB:(qh + 1) * QB, :] = results[c0]["out"] + results[c0 + 1]["out"]
    return outp


def kernel(query, key_, value, Wq, bq, Wk, bk, Wv, bv, Wo, bo):
    nc = _get_nc()
    in_maps = make_in_maps(query, key_, value, Wq, bq, Wk, bk, Wv, bv, Wo, bo)
    res = run_bass_kernel_spmd(nc, in_maps, core_ids=list(range(NCORES)))
    return assemble(res.results)


# revision 17
# speedup vs baseline: 1.5916x; 1.2952x over previous
# Multi-head attention kernel for Trainium2, sharded over 8 NeuronCores.
#
# Sharding: core = (batch b, query-half qh, head-half hh). Each core handles
# 6 heads (3 head-pairs) x 1024 queries of one batch, computing K/V
# projections only for its own 6 heads (no cross-core recompute). The output
# projection is a PARTIAL sum over the core's 6 heads; the two head-half
# partners' partials are summed on the HOST during assembly (exact fp32 add),
# so no on-chip collective is needed.
#
# Engine balance (per core): ScalarE exp stream = 96 x [128,1024] ~= 128us is
# the kernel floor; PE matmul work ~= 110us fits underneath it. The emission
# order software-pipelines everything against the exp stream:
#   - attention runs as 6 "units" (head-pair x query-512-chunk) x 16 key
#     tiles; scores (2 heads row-packed in PE quadrants) -> exp -> PV with
#     a ones-column producing softmax denominators for free.
#   - all projection work (q/k/v) is emitted as deadline-ordered FILLER
#     chunks interleaved into the attention steps, so the PE chews
#     projections inside the exp windows and the HAM clock never idles.
#   - per-unit softmax normalization: DVE reciprocal at partition 64 of the
#     staged PV output, then a K=1 broadcast matmul from partition 64
#     (selector row of ones) and a DVE multiply. No DMA gathers, no batched
#     tail.
#   - output projection (K=128 per head pair, accumulated over 3 pairs) runs
#     in a short tail reusing the score-PSUM slots.
# PSUM budget: scores 2x[128,2,512] (4 banks) + PV accumulators 2x[65,512]
# (2 banks) + projection scratch 2x[128,512] (2 banks) = 8 banks exactly.

import numpy as np
import os
from contextlib import ExitStack

_DEBUG = os.environ.get('KDEBUG', '0') == '1'

import concourse.bass as bass
import concourse.mybir as mybir
import concourse.tile as tile
from concourse import bacc
from concourse.bass_utils import run_bass_kernel_spmd

F32 = mybir.dt.float32
BF16 = mybir.dt.bfloat16
P = 128
E = 768
S = 2048
B = 2
H = 12
D = 64
NCORES = 8
EC = E // P        # 6 e-chunks (contraction over hidden)
KT = S // P        # 16 key tiles
HL = 6             # heads per core
MT = HL * D // P   # 3 m-tiles (head pairs) per core
QB = 1024          # queries per core
QC = 2             # query 512-chunks per core
NC4 = S // 512     # 4 n-slices of k^T


def build_nc():
    nc = bacc.Bacc("TRN2", debug=False)

    # DRAM I/O (per-core shapes; same NEFF on all 8 cores)
    # all inputs pre-arranged on host into on-chip [128-partition, ...] layout
    # so every DMA is a contiguous full-bandwidth copy
    xq = nc.dram_tensor("xq", (QC, P, EC * 512), BF16, kind="ExternalInput")
    xk = nc.dram_tensor("xk", (NC4, P, EC * 512), BF16, kind="ExternalInput")
    xv = nc.dram_tensor("xv", (KT, P, EC * P), BF16, kind="ExternalInput")
    wq = nc.dram_tensor("wq", (MT, P, EC * P), BF16, kind="ExternalInput")  # pre-scaled 1/sqrt(D)
    wk = nc.dram_tensor("wk", (MT, P, EC * P), BF16, kind="ExternalInput")
    wv = nc.dram_tensor("wv", (P, EC * HL * D), BF16, kind="ExternalInput")
    wo = nc.dram_tensor("wo", (P, MT * E), BF16, kind="ExternalInput")
    bq = nc.dram_tensor("bq", (P, MT), F32, kind="ExternalInput")     # per-partition bias per m-tile
    bk = nc.dram_tensor("bk", (P, MT), F32, kind="ExternalInput")
    bo = nc.dram_tensor("bo", (P, E), F32, kind="ExternalInput")      # partial (bv@Wo [+ bo]), broadcast
    out = nc.dram_tensor("out", (QB, E), F32, kind="ExternalOutput")  # PARTIAL over this core's heads
    dbg = None
    if _DEBUG:
        dbg = nc.dram_tensor("dbg", (D, HL * QC * 512), F32, kind="ExternalOutput")

    with tile.TileContext(nc) as tc:
        with ExitStack() as ctx:
            _emit(ctx, tc, nc, xq, xk, xv, wq, wk, wv, wo, bq, bk, bo, out, dbg)
    nc.compile()
    return nc


def _emit(ctx, tc, nc, xq, xk, xv, wq, wk, wv, wo, bq, bk, bo, out, dbg=None):
    # ---- pools ----
    persist = ctx.enter_context(tc.tile_pool(name="persist", bufs=1))
    wpool = ctx.enter_context(tc.tile_pool(name="wpool", bufs=2))
    xvpool = ctx.enter_context(tc.tile_pool(name="xvpool", bufs=3))
    epool = ctx.enter_context(tc.tile_pool(name="epool", bufs=8))
    outpool = ctx.enter_context(tc.tile_pool(name="outpool", bufs=2))
    # PSUM pools: 4 + 2 + 2 = 8 banks
    psC = ctx.enter_context(tc.tile_pool(name="psC", bufs=2, space="PSUM"))    # scores [128,2,512]
    opool = ctx.enter_context(tc.tile_pool(name="opool", bufs=2, space="PSUM"))  # PV accum [65,512]
    pj = ctx.enter_context(tc.tile_pool(name="pj", bufs=2, space="PSUM"))      # proj scratch [128,512]

    # ---- persistent SBUF tensors ----
    qT = persist.tile([P, MT, QB], BF16)          # q^T [384, 1024]
    kT = persist.tile([P, MT, S], BF16)           # k^T [384, 2048]
    v_sb = persist.tile([P, KT, HL, D + 1], BF16)  # v + ones column per head
    o_all = persist.tile([P, MT, QB], BF16)       # normalized o^T, pairs in partition halves
    o_raw = persist.tile([D, HL, QC, 512], F32)   # staged unnormalized o^T
    dens0 = persist.tile([1, 2, 512], F32)        # denominators relocated to partition 0
    drecf = persist.tile([1, 2, 512], F32)        # fast-reciprocal output (partition 0)
    drecb = persist.tile([1, 2, 512], BF16)       # bf16 copy feeding the bc matmul
    sel0 = persist.tile([1, D], BF16)             # row of ones (K=1 broadcast selector)
    bq_sb = persist.tile([P, MT], F32)
    bk_sb = persist.tile([P, MT], F32)
    bo_sb = persist.tile([P, E], F32)
    xq_t = persist.tile([P, QC, EC, 512], BF16)   # resident query slices
    xk_t = persist.tile([P, NC4, EC, 512], BF16)  # resident key slices
    wq_t = persist.tile([P, MT, EC, P], BF16)
    wk_t = persist.tile([P, MT, EC, P], BF16)

    # ---- DMAs: three queues (sync ~200GB/s, scalar ~150, gpsimd ~70),
    # first-needed-first per queue; the first q/k slices are split across
    # two queues so the exp stream can start ~16us in.
    HK = EC // 2 * 512  # half a slice in flat columns
    nc.sync.dma_start(xq_t[:, 0, 0:3, :].rearrange("p a b -> p (a b)"), xq[0][:, 0:HK])
    nc.scalar.dma_start(wq_t[:, 0, :, :].rearrange("p a b -> p (a b)"), wq[0])
    nc.scalar.dma_start(wk_t[:, 0, :, :].rearrange("p a b -> p (a b)"), wk[0])
    nc.gpsimd.dma_start(bq_sb[:], bq[:])
    nc.gpsimd.dma_start(bk_sb[:], bk[:])
    nc.sync.dma_start(xk_t[:, 0, :, :].rearrange("p a b -> p (a b)"), xk[0])
    nc.scalar.dma_start(xq_t[:, 0, 3:6, :].rearrange("p a b -> p (a b)"), xq[0][:, HK:])
    wv_t = wpool.tile([P, EC, HL * D], BF16, tag="w")
    nc.scalar.dma_start(wv_t[:].rearrange("p a b -> p (a b)"), wv[:])
    nc.sync.dma_start(xk_t[:, 1, :, :].rearrange("p a b -> p (a b)"), xk[1])
    nc.scalar.dma_start(wq_t[:, 1, :, :].rearrange("p a b -> p (a b)"), wq[1])
    nc.scalar.dma_start(wk_t[:, 1, :, :].rearrange("p a b -> p (a b)"), wk[1])
    nc.sync.dma_start(xk_t[:, 2, :, :].rearrange("p a b -> p (a b)"), xk[2])
    nc.scalar.dma_start(wq_t[:, 2, :, :].rearrange("p a b -> p (a b)"), wq[2])
    nc.scalar.dma_start(wk_t[:, 2, :, :].rearrange("p a b -> p (a b)"), wk[2])
    nc.sync.dma_start(xk_t[:, 3, :, :].rearrange("p a b -> p (a b)"), xk[3])

    def emit_xq1():
        nc.sync.dma_start(xq_t[:, 1, :, :].rearrange("p a b -> p (a b)"), xq[1])

    def emit_bo():
        nc.sync.dma_start(bo_sb[:], bo[:])

    # constants: ones column for denominators, selector row for broadcast
    nc.vector.memset(v_sb[:, :, :, D], 1.0)
    nc.vector.memset(sel0[:], 1.0)

    # ---- projection emitters (interleaved as filler work) ----
    def emit_q(mt, qc):
        ps = pj.tile([P, 512], F32, tag="pj")
        for ec in range(EC):
            nc.tensor.matmul(ps[:], wq_t[:, mt, ec, :], xq_t[:, qc, ec, :],
                             start=(ec == 0), stop=(ec == EC - 1))
        nc.vector.tensor_scalar_add(qT[:, mt, qc * 512:(qc + 1) * 512], ps[:], bq_sb[:, mt:mt + 1])

    def emit_k(mt, n4):
        ps = pj.tile([P, 512], F32, tag="pj")
        for ec in range(EC):
            nc.tensor.matmul(ps[:], wk_t[:, mt, ec, :], xk_t[:, n4, ec, :],
                             start=(ec == 0), stop=(ec == EC - 1))
        nc.vector.tensor_scalar_add(kT[:, mt, n4 * 512:(n4 + 1) * 512], ps[:], bk_sb[:, mt:mt + 1])

    def emit_v(kt):
        xv_t = xvpool.tile([P, EC, P], BF16, tag="xv")
        eng = nc.gpsimd if kt < 8 else nc.sync
        eng.dma_start(xv_t[:].rearrange("p a b -> p (a b)"), xv[kt])
        ps = pj.tile([P, 512], F32, tag="pj")  # only 384 used
        for ec in range(EC):
            nc.tensor.matmul(ps[:, 0:HL * D], xv_t[:, ec, :], wv_t[:, ec, :],
                             start=(ec == 0), stop=(ec == EC - 1))
        nc.vector.tensor_copy(v_sb[:, kt, :, 0:D],
                              ps[:, 0:HL * D].rearrange("p (h d) -> p h d", d=D))

    # Deadline-ordered filler queue: (deadline_step, emit_fn). Steps run
    # 0..95 (6 units x 16 key tiles). Forced at deadline; otherwise drained
    # at DRAIN_BUDGET items/step to spread PE work under the exp stream.
    fillers = []
    for n4 in range(1, NC4):
        fillers.append((3 * n4 - 1, lambda n4=n4: emit_k(0, n4)))     # unit0 kt=4n4 needs slice n4
    for kt in range(1, KT):
        # v(kt) MUST be emitted before PV(unit0, kt) reads v_sb[kt] — Tile
        # deps follow emission order, so a late v-write would be ordered
        # AFTER the stale read. PV trails by PVT steps, giving kt+PVT-1.
        fillers.append((kt + 2, lambda kt=kt: emit_v(kt)))
    # qc-major unit order: u1=(p1,qc0) at step 16, u2=(p2,qc0) at 32,
    # qc1 units at 48/64/80.
    fillers.append((10, lambda: emit_q(1, 0)))
    for n4 in range(NC4):
        fillers.append((11 + n4, lambda n4=n4: emit_k(1, n4)))
    fillers.append((26, lambda: emit_q(2, 0)))
    for n4 in range(NC4):
        fillers.append((27 + n4, lambda n4=n4: emit_k(2, n4)))
    fillers.append((30, emit_xq1))
    fillers.append((40, emit_bo))
    fillers.append((44, lambda: emit_q(0, 1)))
    fillers.append((58, lambda: emit_q(1, 1)))
    fillers.append((74, lambda: emit_q(2, 1)))
    fillers.sort(key=lambda t: t[0])
    fidx = [0]

    def drain_fillers(step, budget):
        n = 0
        while fidx[0] < len(fillers) and (fillers[fidx[0]][0] <= step or n < budget):
            fillers[fidx[0]][1]()
            fidx[0] += 1
            n += 1

    # ---- prologue: minimal work before the exp stream starts ----
    emit_q(0, 0)
    emit_k(0, 0)
    emit_v(0)

    # ---- attention units: qc-major (all qc0 pairs first) so the qc0
    # output projection can run interleaved into the qc1 units.
    # PV trails the exp stream by PVT steps so v-projection fillers keep
    # lower PE priority than scores and the exp cadence never breaks.
    def make_norm(pair, qc, o_ps):
        def norm():
            for i in range(2):
                h = 2 * pair + i
                nc.vector.tensor_copy(o_raw[:, h, qc, :], o_ps[i][0:D, :])  # stage; frees PSUM
                # denom row partition-shifted 64 -> 0 (approx_fast recip is
                # only correct at base partition 0)
                nc.vector.tensor_copy(dens0[:, i, :], o_ps[i][D:D + 1, :])
            for i in range(2):
                nc.vector.reciprocal_approx_fast(drecf[:, i, :], dens0[:, i, :])
            nc.vector.tensor_copy(drecb[:], drecf[:])
            for i in range(2):
                h = 2 * pair + i
                bc = pj.tile([D, 512], F32, tag="pj", name=f"bc{i}")
                nc.tensor.matmul(bc[:], sel0[:], drecb[:, i, :],
                                 start=True, stop=True)
                nc.vector.tensor_tensor(o_all[i * D:(i + 1) * D, pair, qc * 512:(qc + 1) * 512],
                                        o_raw[:, h, qc, :], bc[:], mybir.AluOpType.mult)
        return norm

    wo_t = wpool.tile([P, MT, E], BF16, tag="w")

    def emit_wo():
        nc.scalar.dma_start(wo_t[:].rearrange("p a b -> p (a b)"), wo[:])
    fillers.append((34, emit_wo))
    fillers.sort(key=lambda t: t[0])

    def emit_oproj(st8):
        op1 = pj.tile([P, 512], F32, tag="pj", name="op1")
        op2 = pj.tile([P, 256], F32, tag="pj", name="op2")
        for pair in range(MT):
            first, last = (pair == 0), (pair == MT - 1)
            nc.tensor.matmul(op1[:], o_all[:, pair, st8 * P:(st8 + 1) * P],
                             wo_t[:, pair, 0:512], start=first, stop=last)
            nc.tensor.matmul(op2[:], o_all[:, pair, st8 * P:(st8 + 1) * P],
                             wo_t[:, pair, 512:768], start=first, stop=last)
        out_sb = outpool.tile([P, E], F32, tag="osb")
        nc.vector.tensor_tensor(out_sb[:, 0:512], op1[:], bo_sb[:, 0:512],
                                mybir.AluOpType.add)
        nc.vector.tensor_tensor(out_sb[:, 512:768], op2[:], bo_sb[:, 512:768],
                                mybir.AluOpType.add)
        nc.sync.dma_start(out[st8 * P:(st8 + 1) * P, :], out_sb[:])

    PVT = 4
    units = [(0, 0), (1, 0), (2, 0), (0, 1), (1, 1), (2, 1)]  # (pair, qc)
    oproj_at = {(3, 12): 0, (4, 4): 1, (4, 12): 2, (5, 4): 3}
    pending_norm = None
    for u, (pair, qc) in enumerate(units):
        o_ps = {i: opool.tile([D + 1, 512], F32, tag="o", name=f"o{i}") for i in range(2)}
        exq = []
        for kt in range(KT):
            drain_fillers(u * KT + kt, 2)
            if kt == 5 and pending_norm is not None:
                pending_norm()
                pending_norm = None
            if (u, kt) in oproj_at:
                emit_oproj(oproj_at[(u, kt)])
            st = psC.tile([P, 2, 512], F32, tag="sc")
            for i in range(2):
                po = D * i
                nc.tensor.matmul(st[:, i, :],
                                 kT[po:po + D, pair, kt * P:(kt + 1) * P],
                                 qT[po:po + D, pair, qc * 512:(qc + 1) * 512],
                                 start=True, stop=True)
            ex = epool.tile([P, 2, 512], BF16, tag="ex")
            nc.scalar.activation(ex[:, :, :], st[:, :, :], mybir.ActivationFunctionType.Exp)
            exq.append(ex)
            if kt >= PVT:
                kk = kt - PVT
                for i in range(2):
                    nc.tensor.matmul(o_ps[i][:, :], v_sb[:, kk, 2 * pair + i, :],
                                     exq[kk][:, i, :], start=(kk == 0), stop=False)
        for kk in range(KT - PVT, KT):
            for i in range(2):
                nc.tensor.matmul(o_ps[i][:, :], v_sb[:, kk, 2 * pair + i, :],
                                 exq[kk][:, i, :], start=False, stop=(kk == KT - 1))
        pending_norm = make_norm(pair, qc, o_ps)
    pending_norm()

    if dbg is not None:
        nc.sync.dma_start(dbg[:, :], o_raw[:].rearrange("p a b c -> p (a b c)"))

    # ---- remaining output projection (qc1 queries) ----
    for st8 in range(4, QB // P):
        emit_oproj(st8)


_NC_CACHE = None


def _get_nc():
    global _NC_CACHE
    if _NC_CACHE is None:
        _NC_CACHE = build_nc()
    return _NC_CACHE


def make_in_maps(query, key_, value, Wq, bq, Wk, bk, Wv, bv, Wo, bo):
    """Host-side sharding + layout prep. Returns list of 8 input dicts."""
    import ml_dtypes
    BF = ml_dtypes.bfloat16
    query = np.asarray(query, dtype=np.float32)
    key_ = np.asarray(key_, dtype=np.float32)
    value = np.asarray(value, dtype=np.float32)
    scale = np.float32(1.0 / np.sqrt(np.float32(D)))

    Wq = np.asarray(Wq, np.float32)
    Wk = np.asarray(Wk, np.float32)
    Wv = np.asarray(Wv, np.float32)
    Wo = np.asarray(Wo, np.float32)
    bq_f = np.asarray(bq, np.float32)
    bk_f = np.asarray(bk, np.float32)
    bv_f = np.asarray(bv, np.float32)
    bo_f = np.asarray(bo, np.float32)

    def pem(a):
        # [E, M] -> [128p, EC, M] -> flat [128, EC*M]
        E_, m = a.shape
        return np.ascontiguousarray(a.reshape(EC, P, m).transpose(1, 0, 2).reshape(P, EC * m))

    def xslices(a, width):
        # [E, S] -> [S//width, 128, EC*width]
        E_, s = a.shape
        n = s // width
        r = a.reshape(EC, P, n, width).transpose(2, 1, 0, 3)
        return np.ascontiguousarray(r.reshape(n, P, EC * width))

    xk_t = [xslices(key_[b].T, 512).astype(BF) for b in range(B)]
    xv_t = [xslices(value[b].T, P).astype(BF) for b in range(B)]
    xq_t = {}
    for b in range(B):
        for qh in range(2):
            xq_t[(b, qh)] = xslices(query[b, qh * QB:(qh + 1) * QB, :].T, 512).astype(BF)

    per_hh = {}
    for hh in range(2):
        hs = slice(hh * HL, (hh + 1) * HL)
        wq_f = np.transpose(Wq[hs], (1, 0, 2)).reshape(E, HL * D) * scale
        wk_f = np.transpose(Wk[hs], (1, 0, 2)).reshape(E, HL * D)
        wv_f = np.transpose(Wv[hs], (1, 0, 2)).reshape(E, HL * D)
        wo_f = Wo[hh * HL * D:(hh + 1) * HL * D, :]
        # wq/wk: per-m-tile chunks [MT, 128, EC*128]
        wq_c = np.stack([pem(wq_f[:, mt * P:(mt + 1) * P]) for mt in range(MT)]).astype(BF)
        wk_c = np.stack([pem(wk_f[:, mt * P:(mt + 1) * P]) for mt in range(MT)]).astype(BF)
        wv_c = pem(wv_f).astype(BF)
        # wo: [384, 768] -> [128, MT*768], partition p holds row mt*128+p
        wo_c = np.ascontiguousarray(
            wo_f.reshape(MT, P, E).transpose(1, 0, 2).reshape(P, MT * E)).astype(BF)
        bq_p = (bq_f[hs].reshape(HL * D) * scale).reshape(MT, P).T.copy()
        bk_p = bk_f[hs].reshape(HL * D).reshape(MT, P).T.copy()
        # v-bias folded through this core's Wo rows; bo itself only on hh=0
        bo_eff = bv_f[hs].reshape(HL * D) @ wo_f
        if hh == 0:
            bo_eff = bo_eff + bo_f
        per_hh[hh] = dict(
            wq=wq_c, wk=wk_c, wv=wv_c, wo=wo_c, bq=bq_p, bk=bk_p,
            bo=np.tile(bo_eff.reshape(1, E), (P, 1)).astype(np.float32).copy(),
        )

    in_maps = []
    for core in range(NCORES):
        b, qh, hh = core // 4, (core // 2) % 2, core % 2
        m = dict(per_hh[hh])
        m["xq"] = xq_t[(b, qh)]
        m["xk"] = xk_t[b]
        m["xv"] = xv_t[b]
        in_maps.append(m)
    return in_maps


def assemble(results):
    outp = np.empty((B, S, E), dtype=np.float32)
    for b in range(B):
        for qh in range(2):
            c0 = b * 4 + qh * 2
            outp[b, qh * QB:(qh + 1) * QB, :] = results[c0]["out"] + results[c0 + 1]["out"]
    return outp


def kernel(query, key_, value, Wq, bq, Wk, bk, Wv, bv, Wo, bo):
    nc = _get_nc()
    in_maps = make_in_maps(query, key_, value, Wq, bq, Wk, bk, Wv, bv, Wo, bo)
    res = run_bass_kernel_spmd(nc, in_maps, core_ids=list(range(NCORES)))
    return assemble(res.results)
